# revision 5
# baseline (speedup 1.0000x reference)
"""Cross-layer transcoder kernel for Trainium2 (8 NeuronCores, SPMD).

Math (from the reference):
    feats[l] = relu(x[l] @ W_enc[l].T + b_enc[l])          # [B, F] per layer
    recon[j] = sum_{i<=j} feats[i] @ W_dec[i, j] + b_dec[j] # [B, D] per layer

Sharding: the transcoder feature dim F=4096 is split across the 8 cores
(512 features each). Each core encodes its feature slice for all layers and
computes a partial reconstruction for every destination layer; the partials
are summed on the host (the gather/unshard step), where b_dec is also added.

Performance notes (v2):
  - The 7 decoder pairs sourced from layer 0, (0, j) for j=1..7, run as fp8e4
    DoubleRow matmuls (2x PE throughput). Their quantization noise is sized to
    keep total rel err ~1.7e-2 (< 2e-2 gate); everything else stays bf16.
  - To let fp8 and bf16 matmuls share one PSUM accumulation group, feats are
    scaled x32 (in the encode activation) and W_dec x1024 (on the host); the
    host divides the summed partials by 32768.
  - Outputs are written as bf16 (halves output DMA; the early phase of the
    kernel is DMA-saturated).
  - Program order is skewed (enc0, enc1, dec0, enc2, dec1, ...) so each
    decode's weight DMA gets a full extra encode phase of slack.
  - x and W_enc are host-packed so the first encode psum group depends only
    on small, early DMA chunks.
"""

import os

import numpy as np
import ml_dtypes

L = 8          # n_layers
B = 1024       # n_pos
D = 768        # d_model
F = 4096       # d_transcoder
NCORES = 8
FL = F // NCORES   # features per core = 512
P = 128
KD = D // P        # 6  encode contraction chunks
MF = FL // P       # 4  feature chunks per core
MD = D // P        # 6  decode output chunks
NB = B // 512      # 2  position chunks of 512

# bf16 decode pairs: (0,0) plus everything with source i>=1.
PAIRS_BF = [(i, j) for j in range(L) for i in range(j + 1)]
NPAIR_BF = len(PAIRS_BF)    # 36
# fp8 DoubleRow pairs: source 0 into dests 1..7.
PAIRS_F8 = []
NPAIR_F8 = len(PAIRS_F8)    # 0

SCALE_F = 32.0      # feats scale (applied in encode activation)
SCALE_W = 1024.0    # W_dec scale (applied on host)
INV_SCALE = 1.0 / (SCALE_F * SCALE_W)

N_WARMUP = 16

BF16 = ml_dtypes.bfloat16
E4M3 = ml_dtypes.float8_e4m3fn

_PROGRAM = None
LAST_EXEC_NS = None
LAST_RESULTS = None


def _build_program():
    import concourse.bacc as bacc
    import concourse.mybir as mybir
    import concourse.tile as tile

    nc = bacc.Bacc("TRN2", target_bir_lowering=False, debug=False)
    bf = mybir.dt.bfloat16
    f8 = mybir.dt.float8e4
    f32 = mybir.dt.float32
    relu = mybir.ActivationFunctionType.Relu
    DR = mybir.MatmulPerfMode.DoubleRow

    # x packed per (layer, position-half, d-chunk): [P, 512] tiles.
    xT_d = nc.dram_tensor("xT", [L, NB, KD, P, 512], bf, kind="ExternalInput")
    # W_enc packed per (layer, feature-chunk): [P(d), KD*128(f?? no: kd-major d cols)]
    # wem[l, mf, p, kd*128+m] = W_enc[l, mf*128+m, kd*128+p]
    wem_d = nc.dram_tensor("wem", [L, MF, P, KD * P], bf, kind="ExternalInput")
    benc_d = nc.dram_tensor("benc", [L, MF, P, 1], f32, kind="ExternalInput")
    # bf16 decode weights (x1024): per pair, per feature-chunk: [P, D]
    wdec_d = nc.dram_tensor("wdec", [NPAIR_BF, MF, P, D], bf, kind="ExternalInput")
    out_d = nc.dram_tensor("outT", [L, D, B], bf, kind="ExternalOutput")

    with tile.TileContext(nc) as tc:
        with (
            tc.tile_pool(name="feats", bufs=1) as feats_pool,
            tc.tile_pool(name="benc", bufs=1) as benc_pool,
            tc.tile_pool(name="xt", bufs=24) as xt_pool,
            tc.tile_pool(name="wem", bufs=8) as wem_pool,
            tc.tile_pool(name="wdec", bufs=56) as wdec_pool,
            tc.tile_pool(name="outs", bufs=8) as out_pool,
            tc.tile_pool(name="psum", bufs=8, space="PSUM") as psum_pool,
        ):
            # Warm up the tensor engine during the prologue DMA fill so HAM
            # un-throttles (~3.4us of PE busy needed) before real matmuls.
            warm = feats_pool.tile([P, 512], bf, name="warm")
            nc.vector.memset(warm, 0)
            wps = psum_pool.tile([P, 512], f32, name="wps", tag="psum")
            for w in range(N_WARMUP):
                nc.tensor.matmul(
                    wps,
                    lhsT=warm[:, :P],
                    rhs=warm,
                    start=(w == 0),
                    stop=(w == N_WARMUP - 1),
                )

            feats = {}        # (layer, mf) -> [P, B] bf16 tile (x32 scaled)
            feats8 = {}       # kf256 -> [P, 2, B] fp8 tile for source layer 0

            def encode(l):
                xts = {}
                wms = []
                bts = []
                # First psum group needs wem[0] and xt[(0, *)] - issue first.
                wm0 = wem_pool.tile([P, KD * P], bf, name="wm", tag="wm")
                nc.sync.dma_start(wm0[0:64, :], wem_d[l, 0, 0:64])
                nc.sync.dma_start(wm0[64:P, :], wem_d[l, 0, 64:P])
                wms.append(wm0)
                for nb in range(NB):
                    for kd in range(KD):
                        xt = xt_pool.tile([P, 512], bf, name="xt", tag="xt")
                        nc.sync.dma_start(xt[0:64, :], xT_d[l, nb, kd, 0:64])
                        nc.sync.dma_start(xt[64:P, :], xT_d[l, nb, kd, 64:P])
                        xts[(nb, kd)] = xt
                    if nb == 0:
                        for mf in range(1, MF):
                            wm = wem_pool.tile([P, KD * P], bf, name="wm", tag="wm")
                            nc.sync.dma_start(wm[0:64, :], wem_d[l, mf, 0:64])
                            nc.sync.dma_start(wm[64:P, :], wem_d[l, mf, 64:P])
                            wms.append(wm)
                for mf in range(MF):
                    bt = benc_pool.tile([P, 1], f32, name=f"benc_{l}_{mf}")
                    nc.scalar.dma_start(bt, benc_d[l, mf])
                    bts.append(bt)
                for mf in range(MF):
                    ft = feats_pool.tile([P, B], bf, name=f"feat_{l}_{mf}")
                    feats[(l, mf)] = ft
                for nb in range(NB):
                    for mf in range(MF):
                        ft = feats[(l, mf)]
                        ps = psum_pool.tile([P, 512], f32, name="ps", tag="psum")
                        for kd in range(KD):
                            nc.tensor.matmul(
                                ps,
                                lhsT=wms[mf][:, kd * P:(kd + 1) * P],
                                rhs=xts[(nb, kd)],
                                start=(kd == 0),
                                stop=(kd == KD - 1),
                            )
                        nc.scalar.activation(
                            ft[:, nb * 512:(nb + 1) * 512], ps, relu,
                            bias=bts[mf], scale=SCALE_F,
                        )

            def decode(j):
                bf_pairs = [(i, j) for i in range(j + 1)]
                wts = {}
                for (i, jj) in bf_pairs:
                    pidx = PAIRS_BF.index((i, jj))
                    for kf in range(MF):
                        wt = wdec_pool.tile([P, D], bf, name="wd", tag="wd")
                        nc.sync.dma_start(wt, wdec_d[pidx, kf])
                        wts[(i, kf)] = wt
                n_mm = 4 * len(bf_pairs)
                for md in range(MD):
                    for nb in range(NB):
                        ps = psum_pool.tile([P, 512], f32, name="ps", tag="psum")
                        c = 0
                        for (i, jj) in bf_pairs:
                            for kf in range(MF):
                                nc.tensor.matmul(
                                    ps,
                                    lhsT=wts[(i, kf)][:, md * P:(md + 1) * P],
                                    rhs=feats[(i, kf)][:, nb * 512:(nb + 1) * 512],
                                    start=(c == 0),
                                    stop=(c == n_mm - 1),
                                )
                                c += 1
                        ot = out_pool.tile([P, 512], bf, name="ot", tag="ot")
                        nc.vector.tensor_copy(ot, ps)
                        d0 = md * P
                        b0 = nb * 512
                        nc.scalar.dma_start(
                            out_d[j, d0:d0 + 64, b0:b0 + 512], ot[0:64, :]
                        )
                        nc.scalar.dma_start(
                            out_d[j, d0 + 64:d0 + P, b0:b0 + 512], ot[64:P, :]
                        )

            # Skewed schedule: encode runs one layer ahead of decode.
            encode(0)
            encode(1)
            for j in range(L - 2):
                decode(j)
                encode(j + 2)
            decode(L - 2)
            decode(L - 1)

    nc.compile()
    return nc


def _prepare_inputs(x, W_enc, b_enc, W_dec):
    """Host-side shard + pack + cast. Returns in_maps for the 8 cores."""
    # xT[l, nb, kd, p, c] = x[l, nb*512+c, kd*128+p]
    xT = np.ascontiguousarray(
        x.transpose(0, 2, 1).reshape(L, KD, P, NB, 512).transpose(0, 3, 1, 2, 4)
    ).astype(BF16)
    in_maps = []
    for c in range(NCORES):
        s = slice(c * FL, (c + 1) * FL)
        We = W_enc[:, s, :]                        # [L, FL, D]
        # wem[l, mf, p, kd*128+m] = We[l, mf*128+m, kd*128+p]
        wem = np.ascontiguousarray(
            We.transpose(0, 2, 1).reshape(L, KD, P, MF, P).transpose(0, 3, 2, 1, 4)
            .reshape(L, MF, P, KD * P)
        ).astype(BF16)
        benc = np.ascontiguousarray(
            SCALE_F * b_enc[:, s], dtype=np.float32
        ).reshape(L, MF, P, 1)
        wdec = np.empty((NPAIR_BF, MF, P, D), dtype=BF16)
        for pidx, (i, j) in enumerate(PAIRS_BF):
            wdec[pidx] = (SCALE_W * W_dec[i, j, s, :]).astype(BF16).reshape(MF, P, D)
        in_maps.append({"xT": xT, "wem": wem, "benc": benc, "wdec": wdec})
    return in_maps


def kernel(x, W_enc, b_enc, W_dec, b_dec):
    global _PROGRAM, LAST_EXEC_NS, LAST_RESULTS
    from concourse import bass_utils

    x = np.asarray(x)
    W_enc = np.asarray(W_enc)
    b_enc = np.asarray(b_enc)
    W_dec = np.asarray(W_dec)
    b_dec = np.asarray(b_dec)

    if _PROGRAM is None:
        _PROGRAM = _build_program()
    nc = _PROGRAM

    in_maps = _prepare_inputs(x, W_enc, b_enc, W_dec)

    trace = os.environ.get("KERNEL_TRACE", "0") == "1"
    res = bass_utils.run_bass_kernel_spmd(
        nc, in_maps, core_ids=list(range(NCORES)), trace=trace
    )
    LAST_EXEC_NS = res.exec_time_ns
    LAST_RESULTS = res

    acc = np.zeros((L, D, B), dtype=np.float32)
    for r in res.results:
        acc += np.asarray(r["outT"], dtype=np.float32)
    out = acc.transpose(0, 2, 1) * INV_SCALE + b_dec.astype(np.float32)[:, None, :]
    return np.ascontiguousarray(out, dtype=np.float32)


# revision 6
# speedup vs baseline: 1.0908x; 1.0908x over previous
"""Cross-layer transcoder kernel for Trainium2 (8 NeuronCores, SPMD).

Math (from the reference):
    feats[l] = relu(x[l] @ W_enc[l].T + b_enc[l])          # [B, F] per layer
    recon[j] = sum_{i<=j} feats[i] @ W_dec[i, j] + b_dec[j] # [B, D] per layer

Sharding: the transcoder feature dim F=4096 is split across the 8 cores
(512 features each). Each core encodes its feature slice for all layers and
computes a partial reconstruction for every destination layer; the partials
are summed on the host (the gather/unshard step), where b_dec is also added.

Performance notes (v2):
  - The 7 decoder pairs sourced from layer 0, (0, j) for j=1..7, run as fp8e4
    DoubleRow matmuls (2x PE throughput). Their quantization noise is sized to
    keep total rel err ~1.7e-2 (< 2e-2 gate); everything else stays bf16.
  - To let fp8 and bf16 matmuls share one PSUM accumulation group, feats are
    scaled x32 (in the encode activation) and W_dec x1024 (on the host); the
    host divides the summed partials by 32768.
  - Outputs are written as bf16 (halves output DMA; the early phase of the
    kernel is DMA-saturated).
  - Program order is skewed (enc0, enc1, dec0, enc2, dec1, ...) so each
    decode's weight DMA gets a full extra encode phase of slack.
  - x and W_enc are host-packed so the first encode psum group depends only
    on small, early DMA chunks.
"""

import os

import numpy as np
import ml_dtypes

L = 8          # n_layers
B = 1024       # n_pos
D = 768        # d_model
F = 4096       # d_transcoder
NCORES = 8
FL = F // NCORES   # features per core = 512
P = 128
KD = D // P        # 6  encode contraction chunks
MF = FL // P       # 4  feature chunks per core
MD = D // P        # 6  decode output chunks
NB = B // 512      # 2  position chunks of 512

# bf16 decode pairs: (0,0) plus everything with source i>=1.
PAIRS_BF = [(i, j) for j in range(L) for i in range(j + 1)]
NPAIR_BF = len(PAIRS_BF)    # 36
# fp8 DoubleRow pairs: source 0 into dests 1..7.
PAIRS_F8 = []
NPAIR_F8 = len(PAIRS_F8)    # 0

SCALE_F = 32.0      # feats scale (applied in encode activation)
SCALE_W = 1024.0    # W_dec scale (applied on host)
INV_SCALE = 1.0 / (SCALE_F * SCALE_W)

N_WARMUP = 16

BF16 = ml_dtypes.bfloat16
E4M3 = ml_dtypes.float8_e4m3fn

_PROGRAM = None
LAST_EXEC_NS = None
LAST_RESULTS = None


def _build_program():
    import concourse.bacc as bacc
    import concourse.mybir as mybir
    import concourse.tile as tile

    nc = bacc.Bacc("TRN2", target_bir_lowering=False, debug=False)
    bf = mybir.dt.bfloat16
    f8 = mybir.dt.float8e4
    f32 = mybir.dt.float32
    relu = mybir.ActivationFunctionType.Relu
    DR = mybir.MatmulPerfMode.DoubleRow

    # x packed per (layer, position-half, d-chunk): [P, 512] tiles.
    xT_d = nc.dram_tensor("xT", [L, NB, KD, P, 512], bf, kind="ExternalInput")
    # W_enc packed per (layer, feature-chunk): [P(d), KD*128(f?? no: kd-major d cols)]
    # wem[l, mf, p, kd*128+m] = W_enc[l, mf*128+m, kd*128+p]
    wem_d = nc.dram_tensor("wem", [L, MF, P, KD * P], bf, kind="ExternalInput")
    benc_d = nc.dram_tensor("benc", [L, MF, P, 1], f32, kind="ExternalInput")
    # bf16 decode weights (x1024): per pair, per feature-chunk: [P, D]
    wdec_d = nc.dram_tensor("wdec", [NPAIR_BF, MF, P, D], bf, kind="ExternalInput")
    out_d = nc.dram_tensor("outT", [L, D, B], bf, kind="ExternalOutput")

    with tile.TileContext(nc) as tc:
        with (
            tc.tile_pool(name="feats", bufs=1) as feats_pool,
            tc.tile_pool(name="benc", bufs=1) as benc_pool,
            tc.tile_pool(name="xt", bufs=24) as xt_pool,
            tc.tile_pool(name="wem", bufs=8) as wem_pool,
            tc.tile_pool(name="wdec", bufs=60) as wdec_pool,
            tc.tile_pool(name="outs", bufs=8) as out_pool,
            tc.tile_pool(name="psum", bufs=8, space="PSUM") as psum_pool,
        ):
            # Warm up the tensor engine during the prologue DMA fill so HAM
            # un-throttles (~3.4us of PE busy needed) before real matmuls.
            warm = feats_pool.tile([P, 512], bf, name="warm")
            nc.vector.memset(warm, 0)
            wps = psum_pool.tile([P, 512], f32, name="wps", tag="psum")
            for w in range(N_WARMUP):
                nc.tensor.matmul(
                    wps,
                    lhsT=warm[:, :P],
                    rhs=warm,
                    start=(w == 0),
                    stop=(w == N_WARMUP - 1),
                )

            feats = {}        # (layer, mf) -> [P, B] bf16 tile (x32 scaled)
            feats8 = {}       # kf256 -> [P, 2, B] fp8 tile for source layer 0

            def encode(l):
                xts = {}
                wms = []
                bts = []
                # First psum group needs wem[0] and xt[(0, *)] - issue first.
                wm0 = wem_pool.tile([P, KD * P], bf, name="wm", tag="wm")
                nc.sync.dma_start(wm0, wem_d[l, 0])
                wms.append(wm0)
                for nb in range(NB):
                    for kd in range(KD):
                        xt = xt_pool.tile([P, 512], bf, name="xt", tag="xt")
                        nc.sync.dma_start(xt, xT_d[l, nb, kd])
                        xts[(nb, kd)] = xt
                    if nb == 0:
                        for mf in range(1, MF):
                            wm = wem_pool.tile([P, KD * P], bf, name="wm", tag="wm")
                            nc.sync.dma_start(wm, wem_d[l, mf])
                            wms.append(wm)
                for mf in range(MF):
                    bt = benc_pool.tile([P, 1], f32, name=f"benc_{l}_{mf}")
                    nc.scalar.dma_start(bt, benc_d[l, mf])
                    bts.append(bt)
                for mf in range(MF):
                    ft = feats_pool.tile([P, B], bf, name=f"feat_{l}_{mf}")
                    feats[(l, mf)] = ft
                for nb in range(NB):
                    for mf in range(MF):
                        ft = feats[(l, mf)]
                        ps = psum_pool.tile([P, 512], f32, name="ps", tag="psum")
                        for kd in range(KD):
                            nc.tensor.matmul(
                                ps,
                                lhsT=wms[mf][:, kd * P:(kd + 1) * P],
                                rhs=xts[(nb, kd)],
                                start=(kd == 0),
                                stop=(kd == KD - 1),
                            )
                        nc.scalar.activation(
                            ft[:, nb * 512:(nb + 1) * 512], ps, relu,
                            bias=bts[mf], scale=SCALE_F,
                        )

            def decode(j):
                bf_pairs = [(i, j) for i in range(j + 1)]
                wts = {}
                for (i, jj) in bf_pairs:
                    pidx = PAIRS_BF.index((i, jj))
                    for kf in range(MF):
                        wt = wdec_pool.tile([P, D], bf, name="wd", tag="wd")
                        nc.sync.dma_start(wt, wdec_d[pidx, kf])
                        wts[(i, kf)] = wt
                n_mm = 4 * len(bf_pairs)
                for md in range(MD):
                    for nb in range(NB):
                        ps = psum_pool.tile([P, 512], f32, name="ps", tag="psum")
                        c = 0
                        for (i, jj) in bf_pairs:
                            for kf in range(MF):
                                nc.tensor.matmul(
                                    ps,
                                    lhsT=wts[(i, kf)][:, md * P:(md + 1) * P],
                                    rhs=feats[(i, kf)][:, nb * 512:(nb + 1) * 512],
                                    start=(c == 0),
                                    stop=(c == n_mm - 1),
                                )
                                c += 1
                        ot = out_pool.tile([P, 512], bf, name="ot", tag="ot")
                        nc.vector.tensor_copy(ot, ps)
                        nc.scalar.dma_start(
                            out_d[j, md * P:(md + 1) * P, nb * 512:(nb + 1) * 512], ot
                        )

            # Skewed schedule: encode runs one layer ahead of decode.
            encode(0)
            encode(1)
            for j in range(L - 2):
                decode(j)
                encode(j + 2)
            decode(L - 2)
            decode(L - 1)

    nc.compile()
    return nc


def _prepare_inputs(x, W_enc, b_enc, W_dec):
    """Host-side shard + pack + cast. Returns in_maps for the 8 cores."""
    # xT[l, nb, kd, p, c] = x[l, nb*512+c, kd*128+p]
    xT = np.ascontiguousarray(
        x.transpose(0, 2, 1).reshape(L, KD, P, NB, 512).transpose(0, 3, 1, 2, 4)
    ).astype(BF16)
    in_maps = []
    for c in range(NCORES):
        s = slice(c * FL, (c + 1) * FL)
        We = W_enc[:, s, :]                        # [L, FL, D]
        # wem[l, mf, p, kd*128+m] = We[l, mf*128+m, kd*128+p]
        wem = np.ascontiguousarray(
            We.transpose(0, 2, 1).reshape(L, KD, P, MF, P).transpose(0, 3, 2, 1, 4)
            .reshape(L, MF, P, KD * P)
        ).astype(BF16)
        benc = np.ascontiguousarray(
            SCALE_F * b_enc[:, s], dtype=np.float32
        ).reshape(L, MF, P, 1)
        wdec = np.empty((NPAIR_BF, MF, P, D), dtype=BF16)
        for pidx, (i, j) in enumerate(PAIRS_BF):
            wdec[pidx] = (SCALE_W * W_dec[i, j, s, :]).astype(BF16).reshape(MF, P, D)
        in_maps.append({"xT": xT, "wem": wem, "benc": benc, "wdec": wdec})
    return in_maps


def kernel(x, W_enc, b_enc, W_dec, b_dec):
    global _PROGRAM, LAST_EXEC_NS, LAST_RESULTS
    from concourse import bass_utils

    x = np.asarray(x)
    W_enc = np.asarray(W_enc)
    b_enc = np.asarray(b_enc)
    W_dec = np.asarray(W_dec)
    b_dec = np.asarray(b_dec)

    if _PROGRAM is None:
        _PROGRAM = _build_program()
    nc = _PROGRAM

    in_maps = _prepare_inputs(x, W_enc, b_enc, W_dec)

    trace = os.environ.get("KERNEL_TRACE", "0") == "1"
    res = bass_utils.run_bass_kernel_spmd(
        nc, in_maps, core_ids=list(range(NCORES)), trace=trace
    )
    LAST_EXEC_NS = res.exec_time_ns
    LAST_RESULTS = res

    acc = np.zeros((L, D, B), dtype=np.float32)
    for r in res.results:
        acc += np.asarray(r["outT"], dtype=np.float32)
    out = acc.transpose(0, 2, 1) * INV_SCALE + b_dec.astype(np.float32)[:, None, :]
    return np.ascontiguousarray(out, dtype=np.float32)
